# revision 2
# baseline (speedup 1.0000x reference)
"""Trainium kernel for nn_AttentiveRNNLanguageModel.

Strategy: batch-parallel over 8 NeuronCores (B=8 -> 1 batch row per core).
The device NEFF computes, per core (its batch row b):
    ctxT  = enc_b.T @ g_b.T                      [H, T]
    combT = tanh(Wc1 @ ctxT + Wc2 @ enc_b.T + b) [H, T]
    logits= comb @ emb.T                         [T, V]   (tied decoder)
Matmuls run in bf16 with fp32 PSUM accumulation.  Host does the cheap
sequential glue (embedding gather, LSTM recurrences, attention weights).
"""

import os
import numpy as np
import ml_dtypes

import concourse.bass as bass
import concourse.bacc as bacc
import concourse.mybir as mybir
import concourse.tile as tile
from concourse.bass_utils import run_bass_kernel_spmd

V, B, T, H, P = 32000, 8, 512, 512, 20
NCORES = 8
F32 = mybir.dt.float32
BF16 = mybir.dt.bfloat16
BF = ml_dtypes.bfloat16

_cache = {}


def _build_attn_nc():
    """Per-core NEFF: ctx -> comb -> logits for one batch row."""
    nc = bacc.Bacc(None, target_bir_lowering=False)

    encA = nc.dram_tensor("encA", [T, H], BF16, kind="ExternalInput")    # enc_b
    gT = nc.dram_tensor("gT", [T, T], BF16, kind="ExternalInput")        # g_b.T
    encTb = nc.dram_tensor("encTb", [H, T], BF16, kind="ExternalInput")  # enc_b.T
    w1T = nc.dram_tensor("w1T", [H, H], BF16, kind="ExternalInput")      # W_cat[:, :H].T
    w2T = nc.dram_tensor("w2T", [H, H], BF16, kind="ExternalInput")      # W_cat[:, H:].T
    bcat = nc.dram_tensor("bcat", [128, 4], F32, kind="ExternalInput")   # b_cat[m*128+p]
    embT = nc.dram_tensor("embT", [H, V], BF16, kind="ExternalInput")    # embedding.T
    out = nc.dram_tensor("logits", [T, V], F32, kind="ExternalOutput")

    KC = H // 128   # 4 contraction chunks
    MC = T // 128   # 4 output-row chunks
    # decoder vocab chunks: 62 x 512 + 1 x 256
    nchunks = [(i * 512, 512) for i in range(62)] + [(31744, 256)]

    with tile.TileContext(nc) as tc:
        with (
            tc.tile_pool(name="const", bufs=1) as cpool,
            tc.tile_pool(name="io", bufs=8) as iopool,
            tc.tile_pool(name="outp", bufs=6) as opool,
            tc.tile_pool(name="ps", bufs=8, space="PSUM") as pspool,
        ):
            # ---- resident inputs ----
            enc_sb = cpool.tile([128, KC, H], BF16, tag="enc")     # [t%128, tchunk, h]
            gT_sb = cpool.tile([128, KC, T], BF16, tag="gt")       # [tk%128, tkchunk, tq]
            encT_sb = cpool.tile([128, KC, T], BF16, tag="encT")   # [h%128, hchunk, t]
            w1_sb = cpool.tile([128, KC, H], BF16, tag="w1")
            w2_sb = cpool.tile([128, KC, H], BF16, tag="w2")
            bc_sb = cpool.tile([128, 4], F32, tag="bc")
            for k in range(KC):
                nc.sync.dma_start(enc_sb[:, k, :], encA[k * 128:(k + 1) * 128, :])
                nc.sync.dma_start(gT_sb[:, k, :], gT[k * 128:(k + 1) * 128, :])
                nc.sync.dma_start(encT_sb[:, k, :], encTb[k * 128:(k + 1) * 128, :])
                nc.sync.dma_start(w1_sb[:, k, :], w1T[k * 128:(k + 1) * 128, :])
                nc.sync.dma_start(w2_sb[:, k, :], w2T[k * 128:(k + 1) * 128, :])
            nc.sync.dma_start(bc_sb[:], bcat[:])

            # ---- ctxT = enc.T @ g.T : lhsT=enc [Tk, H], rhs=gT [Tk, Tq] ----
            ctxT_sb = cpool.tile([128, KC, T], BF16, tag="ctxT")   # [h%128, hchunk, t]
            for m in range(KC):
                ps = pspool.tile([128, T], F32, tag="ps")
                for k in range(KC):
                    nc.tensor.matmul(
                        ps[:],
                        enc_sb[:, k, m * 128:(m + 1) * 128],
                        gT_sb[:, k, :],
                        start=(k == 0), stop=(k == KC - 1),
                    )
                nc.vector.tensor_copy(ctxT_sb[:, m, :], ps[:])

            # ---- combT = tanh(Wc1 @ ctxT + Wc2 @ encT + b) ----
            combT_sb = cpool.tile([128, KC, T], BF16, tag="combT")
            for m in range(KC):
                ps = pspool.tile([128, T], F32, tag="ps")
                for k in range(KC):
                    nc.tensor.matmul(
                        ps[:], w1_sb[:, k, m * 128:(m + 1) * 128], ctxT_sb[:, k, :],
                        start=(k == 0), stop=False,
                    )
                for k in range(KC):
                    nc.tensor.matmul(
                        ps[:], w2_sb[:, k, m * 128:(m + 1) * 128], encT_sb[:, k, :],
                        start=False, stop=(k == KC - 1),
                    )
                nc.scalar.activation(
                    combT_sb[:, m, :], ps[:],
                    mybir.ActivationFunctionType.Tanh,
                    bias=bc_sb[:, m:m + 1],
                )

            # ---- logits = comb @ emb.T : lhsT=combT [H, T], rhs=embT [H, V] ----
            for (n0, nw) in nchunks:
                etiles = []
                for k in range(KC):
                    et = iopool.tile([128, 512], BF16, tag="emb")
                    nc.sync.dma_start(et[:, :nw], embT[k * 128:(k + 1) * 128, n0:n0 + nw])
                    etiles.append(et)
                for m in range(MC):
                    ps = pspool.tile([128, 512], F32, tag="ps")
                    for k in range(KC):
                        nc.tensor.matmul(
                            ps[:, :nw],
                            combT_sb[:, k, m * 128:(m + 1) * 128],
                            etiles[k][:, :nw],
                            start=(k == 0), stop=(k == KC - 1),
                        )
                    ot = opool.tile([128, 512], F32, tag="out")
                    nc.vector.tensor_copy(ot[:, :nw], ps[:, :nw])
                    nc.sync.dma_start(out[m * 128:(m + 1) * 128, n0:n0 + nw], ot[:, :nw])

    nc.compile()
    return nc


def _np_lstm(x, Wih, Whh, bih, bhh):
    b, t, _ = x.shape
    hd = Whh.shape[1]
    xg = x.reshape(b * t, -1) @ Wih.T + (bih + bhh)
    xg = xg.reshape(b, t, -1)
    h = np.zeros((b, hd), np.float32)
    c = np.zeros((b, hd), np.float32)
    WhhT = Whh.T.copy()
    hs = np.empty((b, t, hd), np.float32)
    for j in range(t):
        g = xg[:, j] + h @ WhhT
        i, f, gg, o = np.split(g, 4, axis=-1)
        c = _sig(f) * c + _sig(i) * np.tanh(gg)
        h = _sig(o) * np.tanh(c)
        hs[:, j] = h
    return hs


def _sig(x):
    return 1.0 / (1.0 + np.exp(-x))


def kernel(tokens, pad_lengths, embedding, enc_Wih, enc_Whh, enc_bih, enc_bhh,
           pos_Wih, pos_Whh, pos_bih, pos_bhh, W_mu, b_mu, W_sig, b_sig,
           W_cat, b_cat, dec_b):
    tokens = np.asarray(tokens)
    embedding = np.asarray(embedding, np.float32)
    L = np.asarray(pad_lengths, np.float32)

    # ---- host: embedding gather + encoder LSTM + positional net ----
    emb = embedding[tokens]                                    # [B,T,H]
    enc = _np_lstm(emb, np.asarray(enc_Wih, np.float32), np.asarray(enc_Whh, np.float32),
                   np.asarray(enc_bih, np.float32), np.asarray(enc_bhh, np.float32))
    pos = _np_lstm(enc, np.asarray(pos_Wih, np.float32), np.asarray(pos_Whh, np.float32),
                   np.asarray(pos_bih, np.float32), np.asarray(pos_bhh, np.float32))
    mw = np.maximum(pos @ np.asarray(W_mu, np.float32).T + np.asarray(b_mu, np.float32), 0.0)
    sg = _sig(pos @ np.asarray(W_sig, np.float32).T + np.asarray(b_sig, np.float32))[..., 0]

    mu = np.zeros((B, T), np.float32)
    prev = np.zeros((B,), np.float32)
    jj = np.arange(T, dtype=np.float32)
    for j in range(T):
        w = mw[:, j]
        m = w[:, 0] * prev + w[:, 1] / L + w[:, 2] * (j + 1.0) / L
        m = np.maximum(m, j / L)
        mu[:, j] = m
        prev = m

    rel = jj[None, :] / L[:, None]                             # [B,Tk]
    diff = rel[:, None, :] - mu[:, :, None]
    g = np.exp(-diff ** 2 / (2.0 * sg[:, :, None] ** 2 + 0.001))
    g = np.where(np.tril(np.ones((T, T), bool))[None], g, 0.0)
    g = g / np.maximum(g.sum(-1, keepdims=True), 1e-12)

    # ---- device: ctx/comb/decoder per batch row ----
    if "attn" not in _cache:
        _cache["attn"] = _build_attn_nc()
    nc = _cache["attn"]

    W_cat = np.asarray(W_cat, np.float32)
    w1T = np.ascontiguousarray(W_cat[:, :H].T).astype(BF)
    w2T = np.ascontiguousarray(W_cat[:, H:].T).astype(BF)
    bcat = np.ascontiguousarray(np.asarray(b_cat, np.float32).reshape(4, 128).T)
    embT = np.ascontiguousarray(embedding.T).astype(BF)

    in_maps = []
    for b in range(B):
        in_maps.append({
            "encA": enc[b].astype(BF),
            "gT": np.ascontiguousarray(g[b].T).astype(BF),
            "encTb": np.ascontiguousarray(enc[b].T).astype(BF),
            "w1T": w1T, "w2T": w2T, "bcat": bcat, "embT": embT,
        })

    res = run_bass_kernel_spmd(nc, in_maps, core_ids=list(range(NCORES)))
    globals()["LAST_RESULTS"] = res
    logits = np.stack([res.results[b]["logits"] for b in range(B)], axis=0)
    logits = logits + np.asarray(dec_b, np.float32)[None, None, :]
    return logits.astype(np.float32)



# revision 3
# speedup vs baseline: 1.6811x; 1.6811x over previous
"""Trainium kernel for nn_AttentiveRNNLanguageModel.

Strategy: vocab-sharded tied decoder across 8 NeuronCores.  The decoder
GEMM  logits = comb @ embedding.T  ([B*T,H] @ [H,V]) is 134 GFLOP — ~97%
of the model's compute — and is sharded over the vocab dim: core c
computes logits[:, c*V/8 : (c+1)*V/8] from the full comb and its
embedding slice.  Everything a core needs (comb 4MB + emb slice 4MB,
bf16) stays SBUF-resident, so the tensor engine runs 1024 back-to-back
matmuls with no DMA waits; bf16 logits are staged in SBUF and written
out in 1MB DMAs.

Host does the cheap sequential glue (embedding gather, the two LSTM
recurrences, attention weights, ctx = g@enc, comb = tanh([ctx,enc]@Wc))
— ~24 GFLOP of BLAS-friendly work vs 134 GFLOP on the 8 cores.
"""

import numpy as np
import ml_dtypes

import concourse.bass as bass
import concourse.bacc as bacc
import concourse.mybir as mybir
import concourse.tile as tile
from concourse.bass_utils import run_bass_kernel_spmd

V, B, T, H, P = 32000, 8, 512, 512, 20
NCORES = 8
F32 = mybir.dt.float32
BF16 = mybir.dt.bfloat16
BF = ml_dtypes.bfloat16

BT = B * T          # 4096 output rows (all batch x time)
MC = BT // 128      # 32 row blocks
KC = H // 128       # 4 contraction chunks
NV = V // NCORES    # 4000 vocab cols per core
NW = 500            # vocab cols per matmul (one PSUM bank: 500 fp32 = 2000B)
NC_ = NV // NW      # 8 vocab chunks

_cache = {}


def _build_dec_nc():
    """Per-core NEFF: logits_c = comb @ emb_c.T for this core's vocab slice."""
    nc = bacc.Bacc(None, target_bir_lowering=False)

    # combr[m, p, k, tl] = comb[m*128+tl, k*128+p]   (lhsT chunks, per m-block)
    combr = nc.dram_tensor("combr", [MC, 128, KC, 128], BF16, kind="ExternalInput")
    # embs[p, k, v] = embedding[c*NV+v, k*128+p]     (rhs, this core's slice)
    embs = nc.dram_tensor("embs", [128, KC, NV], BF16, kind="ExternalInput")
    out = nc.dram_tensor("logits", [BT, NV], BF16, kind="ExternalOutput")

    with tile.TileContext(nc) as tc:
        with (
            tc.tile_pool(name="const", bufs=1) as cpool,
            tc.tile_pool(name="stage", bufs=3) as stpool,
            tc.tile_pool(name="ps", bufs=8, space="PSUM") as pspool,
        ):
            # ---- resident inputs ----
            cb_sb = cpool.tile([128, MC, KC, 128], BF16, tag="cb")   # 4MB
            eb_sb = cpool.tile([128, KC, NV], BF16, tag="eb")        # 4MB
            # first m-block's weights, then the embedding slice (needed by
            # m=0 across all n), then the remaining weights.
            nc.sync.dma_start(cb_sb[:, 0], combr[0])
            for n in range(NC_):
                nc.sync.dma_start(eb_sb[:, :, n * NW:(n + 1) * NW],
                                  embs[:, :, n * NW:(n + 1) * NW])
            for m in range(1, MC):
                nc.sync.dma_start(cb_sb[:, m], combr[m])

            # ---- logits_c[m-block] = comb[m-block] @ emb_c.T ----
            for m in range(MC):
                st = stpool.tile([128, NV], BF16, tag="st")
                for n in range(NC_):
                    ps = pspool.tile([128, NW], F32, tag="ps")
                    for k in range(KC):
                        nc.tensor.matmul(
                            ps[:],
                            cb_sb[:, m, k, :],
                            eb_sb[:, k, n * NW:(n + 1) * NW],
                            start=(k == 0), stop=(k == KC - 1),
                        )
                    nc.vector.tensor_copy(st[:, n * NW:(n + 1) * NW], ps[:])
                nc.sync.dma_start(out[m * 128:(m + 1) * 128, :], st[:])

    nc.compile()
    return nc


def _np_lstm(x, Wih, Whh, bih, bhh):
    b, t, _ = x.shape
    hd = Whh.shape[1]
    xg = x.reshape(b * t, -1) @ Wih.T + (bih + bhh)
    xg = xg.reshape(b, t, -1)
    h = np.zeros((b, hd), np.float32)
    c = np.zeros((b, hd), np.float32)
    WhhT = Whh.T.copy()
    hs = np.empty((b, t, hd), np.float32)
    for j in range(t):
        g = xg[:, j] + h @ WhhT
        i, f, gg, o = np.split(g, 4, axis=-1)
        c = _sig(f) * c + _sig(i) * np.tanh(gg)
        h = _sig(o) * np.tanh(c)
        hs[:, j] = h
    return hs


def _sig(x):
    return 1.0 / (1.0 + np.exp(-x))


def kernel(tokens, pad_lengths, embedding, enc_Wih, enc_Whh, enc_bih, enc_bhh,
           pos_Wih, pos_Whh, pos_bih, pos_bhh, W_mu, b_mu, W_sig, b_sig,
           W_cat, b_cat, dec_b):
    tokens = np.asarray(tokens)
    embedding = np.asarray(embedding, np.float32)
    L = np.asarray(pad_lengths, np.float32)

    # ---- host: embedding gather + encoder LSTM + positional net ----
    emb = embedding[tokens]                                    # [B,T,H]
    enc = _np_lstm(emb, np.asarray(enc_Wih, np.float32), np.asarray(enc_Whh, np.float32),
                   np.asarray(enc_bih, np.float32), np.asarray(enc_bhh, np.float32))
    pos = _np_lstm(enc, np.asarray(pos_Wih, np.float32), np.asarray(pos_Whh, np.float32),
                   np.asarray(pos_bih, np.float32), np.asarray(pos_bhh, np.float32))
    mw = np.maximum(pos @ np.asarray(W_mu, np.float32).T + np.asarray(b_mu, np.float32), 0.0)
    sg = _sig(pos @ np.asarray(W_sig, np.float32).T + np.asarray(b_sig, np.float32))[..., 0]

    mu = np.zeros((B, T), np.float32)
    prev = np.zeros((B,), np.float32)
    jj = np.arange(T, dtype=np.float32)
    for j in range(T):
        w = mw[:, j]
        m = w[:, 0] * prev + w[:, 1] / L + w[:, 2] * (j + 1.0) / L
        m = np.maximum(m, j / L)
        mu[:, j] = m
        prev = m

    rel = jj[None, :] / L[:, None]                             # [B,Tk]
    diff = rel[:, None, :] - mu[:, :, None]
    g = np.exp(-diff ** 2 / (2.0 * sg[:, :, None] ** 2 + 0.001))
    g = np.where(np.tril(np.ones((T, T), bool))[None], g, 0.0)
    g = g / np.maximum(g.sum(-1, keepdims=True), 1e-12)

    # ---- host: attention application + combine (cheap GEMMs) ----
    ctx = np.einsum('btk,bkh->bth', g, enc, optimize=True)     # [B,T,H]
    W_cat = np.asarray(W_cat, np.float32)
    comb = np.tanh(ctx.reshape(BT, H) @ W_cat[:, :H].T
                   + enc.reshape(BT, H) @ W_cat[:, H:].T
                   + np.asarray(b_cat, np.float32))            # [BT,H]

    # ---- device: vocab-sharded tied decoder ----
    if "dec" not in _cache:
        _cache["dec"] = _build_dec_nc()
    nc = _cache["dec"]

    # combr[m, p, k, tl] = comb[m*128+tl, k*128+p]
    combr = np.ascontiguousarray(
        comb.reshape(MC, 128, KC, 128).transpose(0, 3, 2, 1)).astype(BF)
    # embs_c[p, k, v] = embedding[c*NV+v, k*128+p]
    embT = embedding.T                                          # [H, V]
    in_maps = []
    for c in range(NCORES):
        esl = embT[:, c * NV:(c + 1) * NV]                      # [H, NV]
        embs = np.ascontiguousarray(
            esl.reshape(KC, 128, NV).transpose(1, 0, 2)).astype(BF)
        in_maps.append({"combr": combr, "embs": embs})

    res = run_bass_kernel_spmd(nc, in_maps, core_ids=list(range(NCORES)))
    globals()["LAST_RESULTS"] = res
    logits = np.concatenate(
        [res.results[c]["logits"].reshape(B, T, NV) for c in range(NCORES)],
        axis=-1).astype(np.float32)
    logits += np.asarray(dec_b, np.float32)[None, None, :]
    return logits


# revision 5
# speedup vs baseline: 1.6854x; 1.0025x over previous
"""Trainium kernel for nn_AttentiveRNNLanguageModel.

Strategy: vocab-sharded tied decoder across 8 NeuronCores.  The decoder
GEMM  logits = comb @ embedding.T  ([B*T,H] @ [H,V]) is 134 GFLOP — ~97%
of the model's compute — and is sharded over the vocab dim: core c
computes logits[:, c*V/8 : (c+1)*V/8] from the full comb and its
embedding slice.  Everything a core needs (comb 4MB + emb slice 4MB,
bf16) stays SBUF-resident, so the tensor engine runs 1024 back-to-back
matmuls with no DMA waits; bf16 logits are staged in SBUF and written
out in 1MB DMAs.

Host does the cheap sequential glue (embedding gather, the two LSTM
recurrences, attention weights, ctx = g@enc, comb = tanh([ctx,enc]@Wc))
— ~24 GFLOP of BLAS-friendly work vs 134 GFLOP on the 8 cores.
"""

import numpy as np
import ml_dtypes

import concourse.bass as bass
import concourse.bacc as bacc
import concourse.mybir as mybir
import concourse.tile as tile
from concourse.bass_utils import run_bass_kernel_spmd

V, B, T, H, P = 32000, 8, 512, 512, 20
NCORES = 8
F32 = mybir.dt.float32
BF16 = mybir.dt.bfloat16
BF = ml_dtypes.bfloat16

BT = B * T          # 4096 output rows (all batch x time)
MC = BT // 128      # 32 row blocks
KC = H // 128       # 4 contraction chunks
NV = V // NCORES    # 4000 vocab cols per core
NW = 500            # vocab cols per matmul (one PSUM bank: 500 fp32 = 2000B)
NC_ = NV // NW      # 8 vocab chunks

_cache = {}


def _build_dec_nc():
    """Per-core NEFF: logits_c = comb @ emb_c.T for this core's vocab slice."""
    nc = bacc.Bacc(None, target_bir_lowering=False)

    # combr[m, p, k, tl] = comb[m*128+tl, k*128+p]   (lhsT chunks, per m-block)
    combr = nc.dram_tensor("combr", [MC, 128, KC, 128], BF16, kind="ExternalInput")
    # embs[p, k, v] = embedding[c*NV+v, k*128+p]     (rhs, this core's slice)
    embs = nc.dram_tensor("embs", [128, KC, NV], BF16, kind="ExternalInput")
    out = nc.dram_tensor("logits", [BT, NV], BF16, kind="ExternalOutput")

    with tile.TileContext(nc) as tc:
        with (
            tc.tile_pool(name="const", bufs=1) as cpool,
            tc.tile_pool(name="stage", bufs=3) as stpool,
            tc.tile_pool(name="ps", bufs=8, space="PSUM") as pspool,
        ):
            # ---- PE warmup: zero matmuls with no DMA deps, running during
            # the input-DMA wait so HAM is at K=8/8 (2.4 GHz) when the real
            # matmuls start (~5us of PE busy flips the 3.4us SHORT window).
            wu_sb = cpool.tile([128, 512], BF16, tag="wu")
            nc.vector.memset(wu_sb[:], 0.0)
            wups = pspool.tile([128, NW], F32, tag="ps")
            for _ in range(12):
                nc.tensor.matmul(wups[:], wu_sb[:, :128], wu_sb[:, :NW],
                                 start=True, stop=True)

            # ---- resident inputs ----
            cb_sb = cpool.tile([128, MC, KC, 128], BF16, tag="cb")   # 4MB
            eb_sb = cpool.tile([128, KC, NV], BF16, tag="eb")        # 4MB
            # load order: first n-chunk of the embedding slice (k-split so
            # the first matmul waits on ~250KB), first m-block's weights,
            # then the rest of the embedding slice, then remaining weights.
            for k in range(KC):
                nc.sync.dma_start(eb_sb[:, k, 0:NW], embs[:, k, 0:NW])
            nc.sync.dma_start(cb_sb[:, 0], combr[0])
            for n in range(1, NC_):
                nc.sync.dma_start(eb_sb[:, :, n * NW:(n + 1) * NW],
                                  embs[:, :, n * NW:(n + 1) * NW])
            for m in range(1, MC):
                nc.sync.dma_start(cb_sb[:, m], combr[m])

            # ---- logits_c[m-block] = comb[m-block] @ emb_c.T ----
            for m in range(MC):
                st = stpool.tile([128, NV], BF16, tag="st")
                for n in range(NC_):
                    ps = pspool.tile([128, NW], F32, tag="ps")
                    for k in range(KC):
                        nc.tensor.matmul(
                            ps[:],
                            cb_sb[:, m, k, :],
                            eb_sb[:, k, n * NW:(n + 1) * NW],
                            start=(k == 0), stop=(k == KC - 1),
                        )
                    nc.vector.tensor_copy(st[:, n * NW:(n + 1) * NW], ps[:])
                    if m == MC - 1 and n % 2 == 1:
                        # last m-block: drain the staged output in 125KB
                        # pieces so the kernel tail isn't one 1MB DMA.
                        nc.sync.dma_start(
                            out[m * 128:(m + 1) * 128,
                                (n - 1) * NW:(n + 1) * NW],
                            st[:, (n - 1) * NW:(n + 1) * NW])
                if m < MC - 1:
                    nc.sync.dma_start(out[m * 128:(m + 1) * 128, :], st[:])

    nc.compile()
    return nc


def _np_lstm(x, Wih, Whh, bih, bhh):
    b, t, _ = x.shape
    hd = Whh.shape[1]
    xg = x.reshape(b * t, -1) @ Wih.T + (bih + bhh)
    xg = xg.reshape(b, t, -1)
    h = np.zeros((b, hd), np.float32)
    c = np.zeros((b, hd), np.float32)
    WhhT = Whh.T.copy()
    hs = np.empty((b, t, hd), np.float32)
    for j in range(t):
        g = xg[:, j] + h @ WhhT
        i, f, gg, o = np.split(g, 4, axis=-1)
        c = _sig(f) * c + _sig(i) * np.tanh(gg)
        h = _sig(o) * np.tanh(c)
        hs[:, j] = h
    return hs


def _sig(x):
    return 1.0 / (1.0 + np.exp(-x))


def kernel(tokens, pad_lengths, embedding, enc_Wih, enc_Whh, enc_bih, enc_bhh,
           pos_Wih, pos_Whh, pos_bih, pos_bhh, W_mu, b_mu, W_sig, b_sig,
           W_cat, b_cat, dec_b):
    tokens = np.asarray(tokens)
    embedding = np.asarray(embedding, np.float32)
    L = np.asarray(pad_lengths, np.float32)

    # ---- host: embedding gather + encoder LSTM + positional net ----
    emb = embedding[tokens]                                    # [B,T,H]
    enc = _np_lstm(emb, np.asarray(enc_Wih, np.float32), np.asarray(enc_Whh, np.float32),
                   np.asarray(enc_bih, np.float32), np.asarray(enc_bhh, np.float32))
    pos = _np_lstm(enc, np.asarray(pos_Wih, np.float32), np.asarray(pos_Whh, np.float32),
                   np.asarray(pos_bih, np.float32), np.asarray(pos_bhh, np.float32))
    mw = np.maximum(pos @ np.asarray(W_mu, np.float32).T + np.asarray(b_mu, np.float32), 0.0)
    sg = _sig(pos @ np.asarray(W_sig, np.float32).T + np.asarray(b_sig, np.float32))[..., 0]

    mu = np.zeros((B, T), np.float32)
    prev = np.zeros((B,), np.float32)
    jj = np.arange(T, dtype=np.float32)
    for j in range(T):
        w = mw[:, j]
        m = w[:, 0] * prev + w[:, 1] / L + w[:, 2] * (j + 1.0) / L
        m = np.maximum(m, j / L)
        mu[:, j] = m
        prev = m

    rel = jj[None, :] / L[:, None]                             # [B,Tk]
    diff = rel[:, None, :] - mu[:, :, None]
    g = np.exp(-diff ** 2 / (2.0 * sg[:, :, None] ** 2 + 0.001))
    g = np.where(np.tril(np.ones((T, T), bool))[None], g, 0.0)
    g = g / np.maximum(g.sum(-1, keepdims=True), 1e-12)

    # ---- host: attention application + combine (cheap GEMMs) ----
    ctx = np.einsum('btk,bkh->bth', g, enc, optimize=True)     # [B,T,H]
    W_cat = np.asarray(W_cat, np.float32)
    comb = np.tanh(ctx.reshape(BT, H) @ W_cat[:, :H].T
                   + enc.reshape(BT, H) @ W_cat[:, H:].T
                   + np.asarray(b_cat, np.float32))            # [BT,H]

    # ---- device: vocab-sharded tied decoder ----
    if "dec" not in _cache:
        _cache["dec"] = _build_dec_nc()
    nc = _cache["dec"]

    # combr[m, p, k, tl] = comb[m*128+tl, k*128+p]
    combr = np.ascontiguousarray(
        comb.reshape(MC, 128, KC, 128).transpose(0, 3, 2, 1)).astype(BF)
    # embs_c[p, k, v] = embedding[c*NV+v, k*128+p]
    embT = embedding.T                                          # [H, V]
    in_maps = []
    for c in range(NCORES):
        esl = embT[:, c * NV:(c + 1) * NV]                      # [H, NV]
        embs = np.ascontiguousarray(
            esl.reshape(KC, 128, NV).transpose(1, 0, 2)).astype(BF)
        in_maps.append({"combr": combr, "embs": embs})

    res = run_bass_kernel_spmd(nc, in_maps, core_ids=list(range(NCORES)))
    globals()["LAST_RESULTS"] = res
    logits = np.concatenate(
        [res.results[c]["logits"].reshape(B, T, NV) for c in range(NCORES)],
        axis=-1).astype(np.float32)
    logits += np.asarray(dec_b, np.float32)[None, None, :]
    return logits
